# revision 62
# baseline (speedup 1.0000x reference)
"""Encoder kernel builder for nn_Encoder (conv stack + segment-mean) on TRN2.
See layout notes in docstring history; key contracts:
  h0_nat [128=(co16*8+dy), 64, 513]; d0_nat [128=(co32*4+dy), 64, 257]
  d1_nat [128=(co64*2+dy), 64, 129]; d2_nat [128, 65, 65] (tl pad)
  d3_nat [128, 2, 33, 33] (br pad);  u0_nat [128, 65, 65] (br pad)
  u1_nat [128=(co64*2+df), 65, 65, 2]; u2_nat [128=(co32*4+df), 65, 129, 2]
  u3_nat [128=(co16*8+df), 64, 260, 2] (X-linear idx = X+4, reflect cols)
  f_nat  [96=(co3*32+dy), 16, 512]
"""
import contextlib
import numpy as np
import ml_dtypes
import concourse.bass as bass
import concourse.tile as tile
from concourse import mybir

BF16 = mybir.dt.bfloat16
F32 = mybir.dt.float32
EPS = 1e-5
AL = mybir.AluOpType
AF = mybir.ActivationFunctionType
P = 512 * 512
_SKIP_SEG_FEED = False


def _bf(x):
    return np.ascontiguousarray(x.astype(ml_dtypes.bfloat16))


def _tc_entry(w, a, b, va, vb):
    ka = a + 1 - 2 * va
    kb = b + 1 - 2 * vb
    if 0 <= ka < 3 and 0 <= kb < 3:
        return w[:, :, ka, kb]
    return None


def pack_inputs(inp):
    out = {}
    x = np.asarray(inp['x'], np.float32)
    xp = np.pad(x, ((0, 0), (3, 3), (3, 3)), mode='reflect')
    out['x_pad'] = _bf(np.pad(xp, ((0, 0), (0, 2), (0, 0))))
    out['ids'] = _bf(np.asarray(inp['instance_map']).reshape(-1).astype(np.float32))

    w = np.asarray(inp['w_in'], np.float32)
    wl0 = np.zeros((16, 21, 128), np.float32)
    for t in range(14):
        for dy in range(8):
            ky = t - dy
            if 0 <= ky < 7:
                wl0[t, :, dy::8] = w[:, :, ky, :].transpose(1, 2, 0).reshape(21, 16)
    # K=84 pack: row (tx, c, u) = wl0[4j+u] row (c*7+tx)
    w84 = np.zeros((4, 84, 128), np.float32)
    for j in range(4):
        for tx in range(7):
            for c in range(3):
                for u in range(4):
                    w84[j, tx * 12 + c * 4 + u, :] = wl0[4 * j + u, c * 7 + tx, :]
    out['w_l0'] = _bf(w84)

    for li, (cin, dyc) in enumerate([(16, 4), (32, 2)]):
        w = np.asarray(inp['w_d%d' % li], np.float32)
        tspan = 2 * (dyc - 1) + 3
        wd = np.zeros((tspan, cin * 3, 128), np.float32)
        for t in range(tspan):
            for dy in range(dyc):
                ky = t - 2 * dy
                if 0 <= ky < 3:
                    wd[t, :, dy::dyc] = w[:, :, ky, :].transpose(1, 2, 0).reshape(
                        cin * 3, 2 * cin)
        out['w_d%d' % li] = _bf(wd)

    w = np.asarray(inp['w_d2'], np.float32)
    wd2 = np.zeros((2, 3, 96, 128), np.float32)
    for h in range(2):
        for t in range(3):
            wd2[h, t] = w[:, 32 * h:32 * h + 32, t, :].transpose(1, 2, 0).reshape(
                96, 128)
    out['w_d2'] = _bf(wd2)

    w = np.asarray(inp['w_d3'], np.float32)
    wd3 = np.zeros((2, 9, 128, 128), np.float32)
    for h in range(2):
        for ky in range(3):
            for kx in range(3):
                wd3[h, ky * 3 + kx] = w[128 * h:128 * h + 128, :, ky, kx].T
    out['w_d3'] = _bf(wd3)

    w = np.asarray(inp['w_u0'], np.float32)
    wu0 = np.zeros((2, 2, 2, 2, 2, 128, 128), np.float32)
    for h in range(2):
        for a in range(2):
            for b in range(2):
                for va in range(2):
                    for vb in range(2):
                        e = _tc_entry(w[128 * h:128 * h + 128], a, b, va, vb)
                        if e is not None:
                            wu0[h, a, b, va, vb] = e
    out['w_u0'] = _bf(wu0)

    w = np.asarray(inp['w_u1'], np.float32)
    wu1 = np.zeros((2, 2, 2, 128, 128), np.float32)
    for b in range(2):
        for t in range(2):
            for hoff in range(2):
                for df in range(2):
                    a = df % 2
                    va = t - (df - a) // 2
                    if va not in (0, 1):
                        continue
                    e = _tc_entry(w, a, b, va, hoff)
                    if e is not None:
                        wu1[b, t, hoff, :, df::2] = e
    out['w_u1'] = _bf(wu1)

    w = np.asarray(inp['w_u2'], np.float32)
    wu2 = np.zeros((2, 2, 3, 64, 128), np.float32)
    for h in range(2):
        for b in range(2):
            for t in range(3):
                for df in range(4):
                    a = df % 2
                    va = t - (df - a) // 2
                    if va not in (0, 1):
                        continue
                    for off in range(2):
                        e = _tc_entry(w[32 * h:32 * h + 32], a, b, va, off)
                        if e is not None:
                            wu2[h, b, t, off::2, df::4] = e
    out['w_u2'] = _bf(wu2)

    w = np.asarray(inp['w_u3'], np.float32)
    wu3 = np.zeros((2, 6, 64, 128), np.float32)
    for b in range(2):
        for t in range(5):
            for df in range(8):
                a = df % 2
                va = t - (df - a) // 2
                if va not in (0, 1):
                    continue
                for off in range(2):
                    e = _tc_entry(w, a, b, va, off)
                    if e is not None:
                        wu3[b, t, off::2, df::8] = e
    wu3p = np.zeros((2, 3, 128, 128), np.float32)
    for b in range(2):
        for j in range(3):
            for u2 in range(2):
                wu3p[b, j, 64 * u2:64 * u2 + 64, :] = wu3[b, 2 * j + u2]
    out['w_u3'] = _bf(wu3p)

    w = np.asarray(inp['w_out'], np.float32)
    wlo = np.zeros((38, 112, 96), np.float32)
    for t in range(38):
        for dy in range(32):
            ky = t - dy
            if 0 <= ky < 7:
                wlo[t, :, dy::32] = w[:, :, ky, :].transpose(1, 2, 0).reshape(112, 3)
    out['w_lo'] = _bf(wlo)
    out['b_lo'] = np.ascontiguousarray(np.repeat(
        np.asarray(inp['b_out'], np.float32), 32)[:, None])

    for name, gs in [('ones8', 8), ('ones4', 4), ('ones2', 2)]:
        m = np.zeros((128, 128), np.float32)
        for i in range(128):
            blk = i // gs
            m[gs * blk:gs * blk + gs, i] = 1.0 / gs
        out[name] = m

    out['iota_tile'] = _bf(np.broadcast_to((np.arange(1024) % 32)[None, :],
                                           (128, 1024)).copy())
    out['iota32'] = (np.arange(128)[:, None] % 32).astype(np.float32)
    ids_i = np.asarray(inp['instance_map']).reshape(-1).astype(np.int64)
    cnt = np.bincount(ids_i, minlength=32).astype(np.float32)
    out['rcnt'] = np.ascontiguousarray(
        (1.0 / np.maximum(cnt, 1.0))[:, None])
    return out


def input_specs():
    return {
        'x_pad': ((3, 520, 518), BF16),
        'ids': ((P,), BF16),
        'w_l0': ((4, 84, 128), BF16),
        'w_d0': ((9, 48, 128), BF16),
        'w_d1': ((5, 96, 128), BF16),
        'w_d2': ((2, 3, 96, 128), BF16),
        'w_d3': ((2, 9, 128, 128), BF16),
        'w_u0': ((2, 2, 2, 2, 2, 128, 128), BF16),
        'w_u1': ((2, 2, 2, 128, 128), BF16),
        'w_u2': ((2, 2, 3, 64, 128), BF16),
        'w_u3': ((2, 3, 128, 128), BF16),
        'w_lo': ((38, 112, 96), BF16),
        'b_lo': ((96, 1), F32),
        'ones8': ((128, 128), F32),
        'ones4': ((128, 128), F32),
        'ones2': ((128, 128), F32),
        'iota_tile': ((128, 1024), BF16),
        'iota32': ((128, 1), F32),
        'rcnt': ((32, 1), F32),
    }


# SBUF weight layouts: (sbuf_shape, einops from DRAM shape)
WSPEC = {
    'w_l0': ((84, 4, 128), "j k m -> k j m"),
    'w_d0': ((48, 9, 128), "t k m -> k t m"),
    'w_d1': ((96, 5, 128), "t k m -> k t m"),
    'w_d2': ((96, 2, 3, 128), "h t k m -> k h t m"),
    'w_d3': ((128, 2, 9, 128), "h t k m -> k h t m"),
    'w_u0': ((128, 2, 2, 2, 2, 2, 128), "h a b va vb k m -> k h a b va vb m"),
    'w_u1': ((128, 2, 2, 2, 128), "b t o k m -> k b t o m"),
    'w_u2': ((64, 2, 2, 3, 128), "h b t k m -> k h b t m"),
    'w_u3': ((128, 2, 3, 128), "b t k m -> k b t m"),
    'w_lo': ((112, 38, 96), "t k m -> k t m"),
    'b_lo': ((96, 1), None),
    'ones8': ((128, 128), None),
    'ones4': ((128, 128), None),
    'ones2': ((128, 128), None),
    'iota_tile': ((128, 1024), None),
    'iota32': ((128, 1), None),
    'rcnt': ((32, 1), None),
}


def _inorm_relu(nc, sm, pp, interior, chunks, ones_lhs):
    """In-place instance-norm + relu. chunks: 2D APs [128, <=512]."""
    nchunk = len(chunks)
    stats = sm.tile([128, nchunk, 6], F32, tag="in_stats")
    for i, ch in enumerate(chunks):
        nc.vector.bn_stats(out=stats[:, i, :], in_=ch)
    mv = sm.tile([128, 2], F32, tag="in_mv")
    nc.vector.bn_aggr(out=mv, in_=stats)
    if ones_lhs is not None:
        m3 = sm.tile([128, 3], F32, tag="in_m3")
        nc.vector.tensor_copy(out=m3[:, 0:2], in_=mv)
        nc.vector.tensor_mul(m3[:, 2:3], mv[:, 0:1], mv[:, 0:1])
        cps = pp.tile([128, 3], F32, tag="in_comb")
        nc.tensor.matmul(cps, lhsT=ones_lhs, rhs=m3, start=True, stop=True)
        mbar = sm.tile([128, 3], F32, tag="in_mbar")
        nc.scalar.copy(out=mbar, in_=cps)
        m_col = mbar[:, 0:1]
        var = sm.tile([128, 1], F32, tag="in_var")
        nc.vector.tensor_add(var, mbar[:, 1:2], mbar[:, 2:3])
        mm = sm.tile([128, 1], F32, tag="in_mm")
        nc.vector.tensor_mul(mm, m_col, m_col)
        nc.vector.tensor_sub(var, var, mm)
    else:
        m_col = mv[:, 0:1]
        var = sm.tile([128, 1], F32, tag="in_var")
        nc.vector.tensor_copy(out=var, in_=mv[:, 1:2])
    nc.vector.tensor_scalar_add(var, var, EPS)
    sd = sm.tile([128, 1], F32, tag="in_sd")
    nc.scalar.sqrt(sd, var)
    s_col = sm.tile([128, 1], F32, tag="in_s")
    nc.vector.reciprocal(s_col, sd)
    b_col = sm.tile([128, 1], F32, tag="in_b")
    nc.vector.tensor_scalar(b_col, m_col, s_col, -1.0, AL.mult, AL.mult)
    nc.scalar.activation(interior, interior, AF.Relu, bias=b_col, scale=s_col)


def build(nc, tc, ctx, upto='seg', dbg=None):
    spec = input_specs()
    din = {k: nc.dram_tensor(k, s, d, kind="ExternalInput")
           for k, (s, d) in spec.items()}
    dbg = dbg or {}
    stages = ['h0', 'd0', 'd1', 'd2', 'd3', 'u0', 'u1', 'u2', 'u3', 'f',
              'sums', 'seg']
    sidx = stages.index(upto)
    out_d = nc.dram_tensor("out", (32, 12, 2048), BF16, kind="ExternalOutput")

    sm = ctx.enter_context(tc.tile_pool(name="small", bufs=2))
    acts = ctx.enter_context(tc.tile_pool(name="acts", bufs=1))
    pp_s = ctx.enter_context(tc.tile_pool(name="psum_s", bufs=1, space="PSUM"))
    wpool = ctx.enter_context(tc.tile_pool(name="weights", bufs=1))
    segp = ctx.enter_context(tc.tile_pool(name="segbig", bufs=1))
    # f4Tx[x%128, xc, 96*grp + c*32 + dy] = f[c, 32*grp+dy, 128*xc+x]
    f4Tx = segp.tile([128, 4, 1536], BF16)
    idsTx = segp.tile([128, 4, 512], BF16)
    ids2d = din['ids'].rearrange("(y x) -> y x", x=512)
    for xc in range(4):
        eng = nc.sync if xc % 2 == 0 else nc.sync
        eng.dma_start(out=idsTx[:, xc, :],
                      in_=ids2d[:, 128 * xc:128 * xc + 128], transpose=True)

    wl_ctr = [0]

    def wload(name, pool=None):
        shape, rs = WSPEC[name]
        t = (pool or wpool).tile(list(shape), spec[name][1], tag="w_" + name)
        src = din[name][:]
        if rs is not None:
            src = src.rearrange(rs)
        eng = nc.sync if wl_ctr[0] % 2 == 0 else nc.sync
        wl_ctr[0] += 1
        eng.dma_start(out=t, in_=src)
        return t

    ones8, ones4, ones2 = wload('ones8'), wload('ones4'), wload('ones2')

    def stage_done(name, tile_ap):
        if name in dbg:
            nc.sync.dma_start(out=dbg[name][:], in_=tile_ap)
        return sidx <= stages.index(name)

    # ================= L0 =================
    h0 = acts.tile([128, 64, 513], BF16, tag="slotA")
    nc.vector.memset(h0[:, :, 0:1], 0.0)
    with tc.tile_pool(name="lp_l0", bufs=3) as hrp, \
         tc.tile_pool(name="pp_l0", bufs=4, space="PSUM") as psp:
        w_l0 = wload('w_l0')
        for gb in range(16):
            # hr84[(tx,c,u), r4, x] = x_pad[c, 32*gb + 4*r4 + u, tx + x]
            hr = hrp.tile([84, 10, 512], BF16, tag="hr")
            for tx in range(7):
                for c in range(3):
                    nc.sync.dma_start(
                        out=hr[12 * tx + 4 * c:12 * tx + 4 * c + 4, :, :],
                        in_=din['x_pad'][c][32 * gb:32 * gb + 40,
                                            tx:tx + 512].rearrange(
                            "(r u) x -> u r x", u=4))
            for g in range(4):
                ps = psp.tile([128, 512], F32, tag="ps")
                for j in range(4):
                    nc.tensor.matmul(ps, lhsT=w_l0[:, j, :],
                                     rhs=hr[:, 2 * g + j, :],
                                     start=(j == 0), stop=(j == 3))
                if g % 2 == 0:
                    nc.scalar.copy(out=h0[:, 4 * gb + g, 1:513], in_=ps)
                else:
                    nc.vector.tensor_copy(out=h0[:, 4 * gb + g, 1:513], in_=ps)
    _inorm_relu(nc, sm, pp_s, h0[:, :, 1:513],
                [h0[:, i, 1:513] for i in range(64)], ones8)
    if stage_done('h0', h0):
        return din

    # ================= D0 =================
    d0 = acts.tile([128, 64, 257], BF16, tag="slotB")
    nc.vector.memset(d0[:, :, 0:1], 0.0)
    h0r = h0.rearrange("(c d) g x -> c d g x", d=8)
    with tc.tile_pool(name="lp_d0", bufs=2) as hrp, \
         tc.tile_pool(name="pp_d0", bufs=4, space="PSUM") as psp:
        w_d0 = wload('w_d0', hrp)
        for gb in range(16):
            hr = hrp.tile([48, 33, 511], BF16, tag="hr")
            hr4 = hr.rearrange("(c t) r x -> t c r x", t=3)
            if gb == 0:
                nc.vector.memset(hr[:, 0:1, :], 0.0)
            for tx in range(3):
                for d in range(8):
                    rr0 = (d + 1) % 8
                    ks = 1 if (gb == 0 and d == 7) else 0
                    rows = [rr0 + 8 * k for k in range(ks, (33 - rr0 + 7) // 8)]
                    g0 = (32 * gb + rows[0] - 1) // 8
                    nc.sync.dma_start(
                        out=hr4[tx][:, rows[0]:rows[-1] + 1:8, :],
                        in_=h0r[:, d, g0:g0 + len(rows), tx:tx + 511])
            for blk in range(2):
                ps = psp.tile([128, 2, 256], F32, tag="ps")
                for t in range(9):
                    s0 = 16 * blk + t
                    rhs = hr[:, s0:s0 + 9:8, 0:511:2]
                    nc.tensor.matmul(ps, lhsT=w_d0[:, t, :], rhs=rhs,
                                     start=(t == 0), stop=(t == 8))
                if blk % 2 == 0:
                    nc.scalar.copy(
                        out=d0[:, 4 * gb + 2 * blk:4 * gb + 2 * blk + 2, 1:257],
                        in_=ps)
                else:
                    nc.vector.tensor_copy(
                        out=d0[:, 4 * gb + 2 * blk:4 * gb + 2 * blk + 2, 1:257],
                        in_=ps)
    _inorm_relu(nc, sm, pp_s, d0[:, :, 1:257],
                [d0[:, i, 1:257] for i in range(64)], ones4)
    if stage_done('d0', d0):
        return din

    # ================= D1 =================
    d1 = acts.tile([128, 64, 129], BF16, tag="slotA")
    nc.vector.memset(d1[:, :, 0:1], 0.0)
    d0r = d0.rearrange("(c d) g x -> c d g x", d=4)
    with tc.tile_pool(name="lp_d1", bufs=2) as hrp, \
         tc.tile_pool(name="pp_d1", bufs=4, space="PSUM") as psp:
        w_d1 = wload('w_d1', hrp)
        for gb in range(8):
            hr = hrp.tile([96, 33, 255], BF16, tag="hr")
            hr4 = hr.rearrange("(c t) r x -> t c r x", t=3)
            if gb == 0:
                nc.vector.memset(hr[:, 0:1, :], 0.0)
            for tx in range(3):
                for d in range(4):
                    rr0 = (d + 1) % 4
                    ks = 1 if (gb == 0 and d == 3) else 0
                    rows = [rr0 + 4 * k for k in range(ks, (33 - rr0 + 3) // 4)]
                    g0 = (32 * gb + rows[0] - 1) // 4
                    eng = nc.sync if (d % 2 == 0) else nc.sync
                    eng.dma_start(
                        out=hr4[tx][:, rows[0]:rows[-1] + 1:4, :],
                        in_=d0r[:, d, g0:g0 + len(rows), tx:tx + 255])
            for blk in range(2):
                ps = psp.tile([128, 4, 128], F32, tag="ps")
                for t in range(5):
                    s0 = 16 * blk + t
                    rhs = hr[:, s0:s0 + 13:4, 0:255:2]
                    nc.tensor.matmul(ps, lhsT=w_d1[:, t, :], rhs=rhs,
                                     start=(t == 0), stop=(t == 4))
                if blk % 2 == 0:
                    nc.scalar.copy(
                        out=d1[:, 8 * gb:8 * gb + 4, 1:129], in_=ps)
                else:
                    nc.vector.tensor_copy(
                        out=d1[:, 8 * gb + 4:8 * gb + 8, 1:129], in_=ps)
    _inorm_relu(nc, sm, pp_s, d1[:, :, 1:129],
                [d1[:, i, 1:129] for i in range(64)], ones2)
    if stage_done('d1', d1):
        return din

    # ================= D2 =================
    d2 = acts.tile([128, 65, 65], BF16, tag="slotB")
    nc.vector.memset(d2[:, 0:1, :], 0.0)
    nc.vector.memset(d2[:, :, 0:1], 0.0)
    d1r = d1.rearrange("(c d) g x -> c d g x", d=2)
    with tc.tile_pool(name="lp_d2", bufs=1) as hrp, \
         tc.tile_pool(name="pp_d2", bufs=4, space="PSUM") as psp:
        w_d2 = wload('w_d2', hrp)
        for gb in range(2):
            hr = hrp.tile([96, 2, 65, 127], BF16, tag="hr")
            hr4 = hr.rearrange("(c t) h r x -> t c h r x", t=3)
            if gb == 0:
                nc.vector.memset(hr[:, :, 0:1, :], 0.0)
            for h in range(2):
                for tx in range(3):
                    for d in range(2):
                        rr0 = (d + 1) % 2
                        ks = 1 if (gb == 0 and d == 1) else 0
                        rows = [rr0 + 2 * k
                                for k in range(ks, (65 - rr0 + 1) // 2)]
                        g0 = (64 * gb + rows[0] - 1) // 2
                        eng = nc.sync if ((h + tx) % 2 == 0) else nc.sync
                        eng.dma_start(
                            out=hr4[tx][:, h, rows[0]:rows[-1] + 1:2, :],
                            in_=d1r[32 * h:32 * h + 32, d, g0:g0 + len(rows),
                                    tx:tx + 127])
            for blk in range(4):
                ps = psp.tile([128, 8, 64], F32, tag="ps")
                first = True
                for h in range(2):
                    for t in range(3):
                        s0 = 16 * blk + t
                        rhs = hr[:, h, s0:s0 + 15:2, 0:127:2]
                        nc.tensor.matmul(ps, lhsT=w_d2[:, h, t, :], rhs=rhs,
                                         start=first, stop=(h == 1 and t == 2))
                        first = False
                if blk % 2 == 0:
                    nc.scalar.copy(
                        out=d2[:, 1 + 32 * gb + 8 * blk:
                               1 + 32 * gb + 8 * blk + 8, 1:65],
                        in_=ps)
                else:
                    nc.vector.tensor_copy(
                        out=d2[:, 1 + 32 * gb + 8 * blk:
                               1 + 32 * gb + 8 * blk + 8, 1:65],
                        in_=ps)
    _inorm_relu(nc, sm, pp_s, d2[:, 1:65, 1:65],
                [d2[:, 1 + i, 1:65] for i in range(64)], None)
    if stage_done('d2', d2):
        return din

    # ================= D3 =================
    d3 = acts.tile([128, 2, 33, 33], BF16, tag="slotA")
    nc.vector.memset(d3[:, :, 32:33, :], 0.0)
    nc.vector.memset(d3[:, :, :, 32:33], 0.0)
    with tc.tile_pool(name="lp_d3", bufs=1) as hrp, \
         tc.tile_pool(name="pp_d3", bufs=4, space="PSUM") as psp:
        w_d3 = wload('w_d3', hrp)
        for h in range(2):
            for blk in range(2):
                ps = psp.tile([128, 16, 32], F32, tag="ps")
                first = True
                for ky in range(3):
                    for kx in range(3):
                        s0 = 32 * blk + ky
                        rhs = d2[:, s0:s0 + 31:2, kx:kx + 63:2]
                        nc.tensor.matmul(ps, lhsT=w_d3[:, h, ky * 3 + kx, :],
                                         rhs=rhs, start=first,
                                         stop=(ky == 2 and kx == 2))
                        first = False
                nc.scalar.copy(out=d3[:, h, 16 * blk:16 * blk + 16, 0:32], in_=ps)
    for h in range(2):
        _inorm_relu(nc, sm, pp_s, d3[:, h, 0:32, 0:32],
                    [d3[:, h, i, 0:32] for i in range(32)], None)
    if stage_done('d3', d3):
        return din

    # ================= U0 =================
    u0 = acts.tile([128, 65, 65], BF16, tag="slotB")
    nc.vector.memset(u0[:, 64:65, :], 0.0)
    nc.vector.memset(u0[:, :, 64:65], 0.0)
    with tc.tile_pool(name="lp_u0", bufs=1) as hrp, \
         tc.tile_pool(name="pp_u0", bufs=4, space="PSUM") as psp:
        w_u0 = wload('w_u0', hrp)
        for a in range(2):
            for b in range(2):
                for blk in range(2):
                    ps = psp.tile([128, 16, 32], F32, tag="ps")
                    mms = [(h, va, vb) for h in range(2) for va in range(2)
                           for vb in range(2)
                           if 0 <= a + 1 - 2 * va < 3 and 0 <= b + 1 - 2 * vb < 3]
                    for mi, (h, va, vb) in enumerate(mms):
                        rhs = d3[:, h, 16 * blk + va:16 * blk + va + 16,
                                 vb:vb + 32]
                        nc.tensor.matmul(ps, lhsT=w_u0[:, h, a, b, va, vb, :],
                                         rhs=rhs, start=(mi == 0),
                                         stop=(mi == len(mms) - 1))
                    nc.scalar.copy(
                        out=u0[:, 32 * blk + a:32 * blk + a + 31:2, b:b + 63:2],
                        in_=ps)
    _inorm_relu(nc, sm, pp_s, u0[:, 0:64, 0:64],
                [u0[:, i, 0:64] for i in range(64)], None)
    if stage_done('u0', u0):
        return din

    # ================= U1 =================
    u1 = acts.tile([128, 65, 65, 2], BF16, tag="slotA")
    nc.vector.memset(u1[:, 64:65, :, :], 0.0)
    nc.vector.memset(u1[:, :, 64:65, :], 0.0)
    with tc.tile_pool(name="lp_u1", bufs=1) as hrp, \
         tc.tile_pool(name="pp_u1", bufs=4, space="PSUM") as psp:
        w_u1 = wload('w_u1', hrp)
        for b in range(2):
            for blk in range(8):
                ps = psp.tile([128, 8, 64], F32, tag="ps")
                mms = [(t, hoff) for t in range(2) for hoff in range(2)]
                for mi, (t, hoff) in enumerate(mms):
                    rhs = u0[:, 8 * blk + t:8 * blk + t + 8, hoff:hoff + 64]
                    nc.tensor.matmul(ps, lhsT=w_u1[:, b, t, hoff, :], rhs=rhs,
                                     start=(mi == 0), stop=(mi == len(mms) - 1))
                if blk % 2 == 0:
                    nc.scalar.copy(out=u1[:, 8 * blk:8 * blk + 8, 0:64, b],
                                   in_=ps)
                else:
                    nc.vector.tensor_copy(
                        out=u1[:, 8 * blk:8 * blk + 8, 0:64, b], in_=ps)
    u1x = u1.rearrange("p g x b -> p g (x b)")
    _inorm_relu(nc, sm, pp_s, u1x[:, 0:64, 0:128],
                [u1x[:, i, 0:128] for i in range(64)], ones2)
    if stage_done('u1', u1):
        return din

    # ================= U2 =================
    u2 = acts.tile([128, 65, 129, 2], BF16, tag="slotB")
    nc.vector.memset(u2[:, 64:65, :, :], 0.0)
    nc.vector.memset(u2[:, :, 128:129, :], 0.0)
    with tc.tile_pool(name="lp_u2", bufs=1) as hrp, \
         tc.tile_pool(name="pp_u2", bufs=4, space="PSUM") as psp:
        w_u2 = wload('w_u2', hrp)
        u1rA = hrp.tile([64, 129, 129], BF16, tag="u1repA")
        u1rB = hrp.tile([64, 129, 129], BF16, tag="u1repB")
        u1rh = [u1rA, u1rB]
        for h in range(2):
            u1rr = u1rh[h].rearrange("(c o) r x -> o c r x", o=2)
            for off in range(2):
                for df in range(2):
                    cnt = 65 if df == 0 else 64
                    eng = nc.sync if ((h + off) % 2 == 0) else nc.sync
                    eng.dma_start(
                        out=u1rr[off, :, df:df + 2 * cnt - 1:2, :],
                        in_=u1x[64 * h + df:64 * h + df + 63:2, 0:cnt,
                                off:off + 129])
        for b in range(2):
            for blk in range(16):
                ps = psp.tile([128, 4, 128], F32, tag="ps")
                first = True
                for h in range(2):
                    for t in range(3):
                        s0 = 8 * blk + t
                        rhs = u1rh[h][:, s0:s0 + 7:2, 0:128]
                        nc.tensor.matmul(ps, lhsT=w_u2[:, h, b, t, :], rhs=rhs,
                                         start=first, stop=(h == 1 and t == 2))
                        first = False
                if blk % 2 == 0:
                    nc.scalar.copy(out=u2[:, 4 * blk:4 * blk + 4, 0:128, b],
                                   in_=ps)
                else:
                    nc.vector.tensor_copy(
                        out=u2[:, 4 * blk:4 * blk + 4, 0:128, b], in_=ps)
    u2x = u2.rearrange("p g x b -> p g (x b)")
    _inorm_relu(nc, sm, pp_s, u2x[:, 0:64, 0:256],
                [u2x[:, i, 0:256] for i in range(64)], ones4)
    if stage_done('u2', u2):
        return din

    # ================= U3 =================
    u3 = acts.tile([128, 64, 260, 2], BF16, tag="slotA")
    u3X = u3.rearrange("p g x b -> p g (x b)")
    w_u3 = wload('w_u3')
    with tc.tile_pool(name="lp_u3", bufs=2) as hrp, \
         tc.tile_pool(name="pp_u3", bufs=4, space="PSUM") as psp:
        u2q = u2x.rearrange("(c d) g x -> c d g x", d=4)
        for gb in range(4):
            # u2r[(u2,c,o), r2, x] = orig row 2*r2+u2 of the (c,o)-repl tile
            u2r = hrp.tile([128, 33, 257], BF16, tag="u2rep")
            nc.vector.memset(u2r[64:128, 32:33, :], 0.0)
            for off in range(2):
                for d in range(4):
                    cnt = (65 - d + 3) // 4
                    u2b = 64 * (d % 2)
                    r0 = d // 2
                    nc.sync.dma_start(
                        out=u2r[u2b + off:u2b + off + 63:2,
                                r0:r0 + 2 * cnt - 1:2, :],
                        in_=u2q[:, d, 16 * gb:16 * gb + cnt, off:off + 257])
            for b in range(2):
                for blkl in range(8):
                    ps = psp.tile([128, 2, 256], F32, tag="ps")
                    for j in range(3):
                        s0 = 4 * blkl + j
                        rhs = u2r[:, s0:s0 + 3:2, 0:256]
                        nc.tensor.matmul(ps, lhsT=w_u3[:, b, j, :], rhs=rhs,
                                         start=(j == 0), stop=(j == 2))
                    g3 = 16 * gb + 2 * blkl
                    if blkl % 2 == 0:
                        nc.scalar.copy(out=u3[:, g3:g3 + 2, 2:258, b], in_=ps)
                    else:
                        nc.vector.tensor_copy(out=u3[:, g3:g3 + 2, 2:258, b],
                                              in_=ps)
    _inorm_relu(nc, sm, pp_s, u3X[:, :, 4:516],
                [u3X[:, i, 4:516] for i in range(64)], ones8)
    for dst, src in [(3, 5), (2, 6), (1, 7), (516, 514), (517, 513), (518, 512)]:
        nc.scalar.copy(out=u3X[:, :, dst:dst + 1], in_=u3X[:, :, src:src + 1])
    if stage_done('u3', u3):
        return din

    # ================= L_out =================
    f_nat = acts.tile([96, 16, 512], BF16, tag="slotB")
    f_d2 = nc.dram_tensor("f_d2", (16, 96, 512), BF16, kind="Internal")
    # u3d[c, y+3, x] = relu(inorm(u3))[c, y, x] with 3-row reflect pads baked
    # in; x dim = u3X cols 1..518 (x-reflect cols already materialized).
    u3d = nc.dram_tensor("u3d", (16, 518, 518), BF16, kind="Internal")
    for c in range(16):
        eng = nc.sync if c % 2 == 0 else nc.sync
        eng.dma_start(
            out=u3d[c][3:515, :].rearrange("(g b) x -> b g x", b=8),
            in_=u3X[8 * c:8 * c + 8, :, 1:519])
    for r, y in [(0, 3), (1, 2), (2, 1), (515, 510), (516, 509), (517, 508)]:
        eng = nc.sync if r % 2 == 0 else nc.sync
        eng.dma_start(out=u3d[:, r, :],
                      in_=u3X[y % 8:y % 8 + 121:8, y // 8, 1:519])
    w_lo = wload('w_lo')
    b_lo = wload('b_lo')
    with tc.tile_pool(name="lp_lo", bufs=2) as hrp, \
         tc.tile_pool(name="pp_lo", bufs=4, space="PSUM") as psp:
        for grp in range(16):
            hrA = hrp.tile([112, 19, 512], BF16, tag="hr")
            hrB = hrp.tile([112, 19, 512], BF16, tag="hr")
            hA4 = hrA.rearrange("(c t) r x -> t c r x", t=7)
            hB4 = hrB.rearrange("(c t) r x -> t c r x", t=7)
            for tx in range(7):
                eng = nc.sync if (grp + tx) % 2 == 0 else nc.sync
                eng.dma_start(
                    out=hA4[tx],
                    in_=u3d[:, 32 * grp:32 * grp + 19, tx:tx + 512])
                eng.dma_start(
                    out=hB4[tx],
                    in_=u3d[:, 32 * grp + 19:32 * grp + 38, tx:tx + 512])
            ps = psp.tile([96, 512], F32, tag="ps")
            for t in range(38):
                rhs = hrA[:, t, :] if t < 19 else hrB[:, t - 19, :]
                nc.tensor.matmul(ps, lhsT=w_lo[:, t, :], rhs=rhs,
                                 start=(t == 0), stop=(t == 37))
            nc.scalar.activation(f_nat[:, grp, :], ps, AF.Tanh, bias=b_lo,
                                 scale=1.0)
            for xc in range(4):
                eng = nc.sync if (grp + xc) % 2 == 0 else nc.sync
                eng.dma_start(
                    out=f4Tx[:, xc, 96 * grp:96 * grp + 96],
                    in_=f_nat[:, grp, 128 * xc:128 * xc + 128],
                    transpose=True)
    if stage_done('f', f_nat):
        return din

    # ================= segment mean =================
    iota_tile = wload('iota_tile', segp)
    iota32 = wload('iota32', segp)
    ohp = ctx.enter_context(tc.tile_pool(name="segoh", bufs=3))
    gat = ctx.enter_context(tc.tile_pool(name="seggat", bufs=3))
    ppg = ctx.enter_context(tc.tile_pool(name="psumg", bufs=4, space="PSUM"))

    rcnt = wload('rcnt', segp)
    if 'f4Tx' in dbg:
        nc.sync.dma_start(out=dbg['f4Tx'][:], in_=f4Tx[:])
    if 'idsTx' in dbg:
        nc.sync.dma_start(out=dbg['idsTx'][:], in_=idsTx[:])
    psum_s2 = pp_s.tile([3, 32], F32, tag="segsum")
    iota_3d = iota_tile[:, 0:1024].rearrange("p (b k) -> p b k", k=32)
    for xc in range(4):
        for yg in range(16):
            oh = ohp.tile([128, 32, 32], BF16)
            nc.vector.tensor_tensor(
                out=oh,
                in0=idsTx[:, xc, 32 * yg:32 * yg + 32].unsqueeze(2)
                .broadcast_to([128, 32, 32]),
                in1=iota_3d, op=AL.is_equal)
            for yi in range(32):
                y = 32 * yg + yi
                base = 96 * (y // 32) + (y % 32)
                nc.tensor.matmul(psum_s2,
                                 lhsT=f4Tx[:, xc, base:base + 65:32],
                                 rhs=oh[:, yi, :],
                                 start=(xc == 0 and y == 0),
                                 stop=(xc == 3 and y == 511))

    sums32 = sm.tile([32, 32], F32, tag="sums32")
    nc.vector.memset(sums32, 0.0)
    nc.scalar.copy(out=sums32[0:3, :], in_=psum_s2)
    sumsT = sm.tile([32, 32], F32, tag="sumsT")
    nc.vector.transpose(sumsT, sums32)
    means_bf = sm.tile([32, 3], BF16, tag="means_bf")
    nc.vector.tensor_scalar_mul(means_bf, sumsT[:, 0:3], rcnt)
    bd = sm.tile([128, 12], BF16, tag="bd")
    nc.vector.memset(bd, 0.0)
    for s in range(4):
        nc.sync.dma_start(out=bd[32 * s:32 * s + 32, 3 * s:3 * s + 3],
                          in_=means_bf)

    if sidx <= stages.index('sums'):
        return din
    ids_q = din['ids'].rearrange("(q n) -> q n", q=4)
    for t in range(32):
        ids_rep = gat.tile([128, 2048], BF16)
        for q in range(4):
            eng = nc.sync if q % 2 == 0 else nc.sync
            eng.dma_start(
                out=ids_rep[32 * q:32 * q + 32, :],
                in_=ids_q[q:q + 1, t * 2048:(t + 1) * 2048].broadcast_to(
                    [32, 2048]))
        oh_g = gat.tile([128, 2048], BF16)
        nc.vector.tensor_scalar(out=oh_g, in0=ids_rep, scalar1=iota32,
                                scalar2=None, op0=AL.is_equal)
        stg = gat.tile([12, 2048], BF16)
        for w in range(4):
            psg = ppg.tile([12, 512], F32)
            nc.tensor.matmul(psg, lhsT=bd, rhs=oh_g[:, 512 * w:512 * w + 512],
                             start=True, stop=True)
            if w % 2 == 0:
                nc.vector.tensor_copy(out=stg[:, 512 * w:512 * w + 512],
                                      in_=psg)
            else:
                nc.scalar.copy(out=stg[:, 512 * w:512 * w + 512], in_=psg)
        nc.sync.dma_start(out=out_d[t], in_=stg)
    return din


# ======================================================================
# public entry: kernel(**inputs) with FULL batch inputs, 8-core SPMD
# ======================================================================
import concourse.bacc as _bacc
from concourse import bass_utils as _bass_utils

_CACHE = {}


def _get_nc():
    if 'nc' not in _CACHE:
        nc = _bacc.Bacc("TRN2", target_bir_lowering=False)
        with contextlib.ExitStack() as ctx:
            tc = ctx.enter_context(tile.TileContext(nc, pool_alloc_mode="queue"))
            build(nc, tc, ctx, upto='seg')
        nc.compile()
        _CACHE['nc'] = nc
    return _CACHE['nc']


def kernel(**inputs):
    nc = _get_nc()
    x = np.asarray(inputs['x'])
    ids = np.asarray(inputs['instance_map'])
    B = x.shape[0]
    shared = None
    in_maps = []
    for bi in range(B):
        inp0 = {k: v for k, v in inputs.items()}
        inp0['x'] = x[bi]
        inp0['instance_map'] = ids[bi]
        if shared is None:
            m = pack_inputs(inp0)
            shared = {k: v for k, v in m.items()
                      if k not in ('x_pad', 'ids', 'rcnt')}
        else:
            m = dict(shared)
            xp = np.pad(np.asarray(inp0['x'], np.float32), ((0, 0), (3, 3), (3, 3)),
                        mode='reflect')
            m['x_pad'] = _bf(np.pad(xp, ((0, 0), (0, 2), (0, 0))))
            ids_i = np.asarray(inp0['instance_map']).reshape(-1)
            m['ids'] = _bf(ids_i.astype(np.float32))
            cnt = np.bincount(ids_i.astype(np.int64),
                              minlength=32).astype(np.float32)
            m['rcnt'] = np.ascontiguousarray(
                (1.0 / np.maximum(cnt, 1.0))[:, None])
        in_maps.append(m)
    res = _bass_utils.run_bass_kernel_spmd(nc, in_maps, core_ids=list(range(B)))
    out = np.stack([_unpack_out(res.results[i]['out']) for i in range(B)])
    return out.astype(np.float32)


def _unpack_out(a):
    a = np.asarray(a).astype(np.float32).reshape(32, 4, 3, 2048)
    return a.transpose(2, 1, 0, 3).reshape(3, 512, 512)


def kernel_traced(**inputs):
    """Like kernel() but with NTFF tracing; returns (out, exec_time_ns, profile)."""
    nc = _get_nc()
    x = np.asarray(inputs['x'])
    ids = np.asarray(inputs['instance_map'])
    B = x.shape[0]
    shared = None
    in_maps = []
    for bi in range(B):
        inp0 = {k: v for k, v in inputs.items()}
        inp0['x'] = x[bi]
        inp0['instance_map'] = ids[bi]
        if shared is None:
            m = pack_inputs(inp0)
            shared = {k: v for k, v in m.items()
                      if k not in ('x_pad', 'ids', 'rcnt')}
        else:
            m = dict(shared)
            xp = np.pad(np.asarray(inp0['x'], np.float32), ((0, 0), (3, 3), (3, 3)),
                        mode='reflect')
            m['x_pad'] = _bf(np.pad(xp, ((0, 0), (0, 2), (0, 0))))
            ids_i = np.asarray(inp0['instance_map']).reshape(-1)
            m['ids'] = _bf(ids_i.astype(np.float32))
            cnt = np.bincount(ids_i.astype(np.int64),
                              minlength=32).astype(np.float32)
            m['rcnt'] = np.ascontiguousarray(
                (1.0 / np.maximum(cnt, 1.0))[:, None])
        in_maps.append(m)
    res = _bass_utils.run_bass_kernel_spmd(nc, in_maps, core_ids=list(range(B)),
                                           trace=True)
    out = np.stack([_unpack_out(res.results[i]['out']) for i in range(B)])
    return out.astype(np.float32), res.exec_time_ns, res



# revision 64
# speedup vs baseline: 1.1555x; 1.1555x over previous
"""Encoder kernel builder for nn_Encoder (conv stack + segment-mean) on TRN2.
See layout notes in docstring history; key contracts:
  h0_nat [128=(co16*8+dy), 64, 513]; d0_nat [128=(co32*4+dy), 64, 257]
  d1_nat [128=(co64*2+dy), 64, 129]; d2_nat [128, 65, 65] (tl pad)
  d3_nat [128, 2, 33, 33] (br pad);  u0_nat [128, 65, 65] (br pad)
  u1_nat [128=(co64*2+df), 65, 65, 2]; u2_nat [128=(co32*4+df), 65, 129, 2]
  u3_nat [128=(co16*8+df), 64, 260, 2] (X-linear idx = X+4, reflect cols)
  f_nat  [96=(co3*32+dy), 16, 512]
"""
import contextlib
import numpy as np
import ml_dtypes
import concourse.bass as bass
import concourse.tile as tile
from concourse import mybir

BF16 = mybir.dt.bfloat16
F32 = mybir.dt.float32
EPS = 1e-5
AL = mybir.AluOpType
AF = mybir.ActivationFunctionType
P = 512 * 512
_SKIP_SEG_FEED = False


def _bf(x):
    return np.ascontiguousarray(x.astype(ml_dtypes.bfloat16))


def _tc_entry(w, a, b, va, vb):
    ka = a + 1 - 2 * va
    kb = b + 1 - 2 * vb
    if 0 <= ka < 3 and 0 <= kb < 3:
        return w[:, :, ka, kb]
    return None


def pack_inputs(inp):
    out = {}
    x = np.asarray(inp['x'], np.float32)
    xp = np.pad(x, ((0, 0), (3, 3), (3, 3)), mode='reflect')
    out['x_pad'] = _bf(np.pad(xp, ((0, 0), (0, 2), (0, 0))))
    out['ids'] = _bf(np.asarray(inp['instance_map']).reshape(-1).astype(np.float32))

    w = np.asarray(inp['w_in'], np.float32)
    wl0 = np.zeros((16, 21, 128), np.float32)
    for t in range(14):
        for dy in range(8):
            ky = t - dy
            if 0 <= ky < 7:
                wl0[t, :, dy::8] = w[:, :, ky, :].transpose(1, 2, 0).reshape(21, 16)
    # K=84 pack: row (tx, c, u) = wl0[4j+u] row (c*7+tx)
    w84 = np.zeros((4, 84, 128), np.float32)
    for j in range(4):
        for tx in range(7):
            for c in range(3):
                for u in range(4):
                    w84[j, tx * 12 + c * 4 + u, :] = wl0[4 * j + u, c * 7 + tx, :]
    out['w_l0'] = _bf(w84)

    for li, (cin, dyc) in enumerate([(16, 4), (32, 2)]):
        w = np.asarray(inp['w_d%d' % li], np.float32)
        tspan = 2 * (dyc - 1) + 3
        wd = np.zeros((tspan, cin * 3, 128), np.float32)
        for t in range(tspan):
            for dy in range(dyc):
                ky = t - 2 * dy
                if 0 <= ky < 3:
                    wd[t, :, dy::dyc] = w[:, :, ky, :].transpose(1, 2, 0).reshape(
                        cin * 3, 2 * cin)
        out['w_d%d' % li] = _bf(wd)

    w = np.asarray(inp['w_d2'], np.float32)
    wd2 = np.zeros((2, 3, 96, 128), np.float32)
    for h in range(2):
        for t in range(3):
            wd2[h, t] = w[:, 32 * h:32 * h + 32, t, :].transpose(1, 2, 0).reshape(
                96, 128)
    out['w_d2'] = _bf(wd2)

    w = np.asarray(inp['w_d3'], np.float32)
    wd3 = np.zeros((2, 9, 128, 128), np.float32)
    for h in range(2):
        for ky in range(3):
            for kx in range(3):
                wd3[h, ky * 3 + kx] = w[128 * h:128 * h + 128, :, ky, kx].T
    out['w_d3'] = _bf(wd3)

    w = np.asarray(inp['w_u0'], np.float32)
    wu0 = np.zeros((2, 2, 2, 2, 2, 128, 128), np.float32)
    for h in range(2):
        for a in range(2):
            for b in range(2):
                for va in range(2):
                    for vb in range(2):
                        e = _tc_entry(w[128 * h:128 * h + 128], a, b, va, vb)
                        if e is not None:
                            wu0[h, a, b, va, vb] = e
    out['w_u0'] = _bf(wu0)

    w = np.asarray(inp['w_u1'], np.float32)
    wu1 = np.zeros((2, 2, 2, 128, 128), np.float32)
    for b in range(2):
        for t in range(2):
            for hoff in range(2):
                for df in range(2):
                    a = df % 2
                    va = t - (df - a) // 2
                    if va not in (0, 1):
                        continue
                    e = _tc_entry(w, a, b, va, hoff)
                    if e is not None:
                        wu1[b, t, hoff, :, df::2] = e
    out['w_u1'] = _bf(wu1)

    w = np.asarray(inp['w_u2'], np.float32)
    wu2 = np.zeros((2, 2, 3, 64, 128), np.float32)
    for h in range(2):
        for b in range(2):
            for t in range(3):
                for df in range(4):
                    a = df % 2
                    va = t - (df - a) // 2
                    if va not in (0, 1):
                        continue
                    for off in range(2):
                        e = _tc_entry(w[32 * h:32 * h + 32], a, b, va, off)
                        if e is not None:
                            wu2[h, b, t, off::2, df::4] = e
    out['w_u2'] = _bf(wu2)

    w = np.asarray(inp['w_u3'], np.float32)
    wu3 = np.zeros((2, 5, 64, 128), np.float32)
    for b in range(2):
        for t in range(5):
            for df in range(8):
                a = df % 2
                va = t - (df - a) // 2
                if va not in (0, 1):
                    continue
                for off in range(2):
                    e = _tc_entry(w, a, b, va, off)
                    if e is not None:
                        wu3[b, t, off::2, df::8] = e
    out['w_u3'] = _bf(wu3)

    w = np.asarray(inp['w_out'], np.float32)
    wlo = np.zeros((38, 112, 96), np.float32)
    for t in range(38):
        for dy in range(32):
            ky = t - dy
            if 0 <= ky < 7:
                wlo[t, :, dy::32] = w[:, :, ky, :].transpose(1, 2, 0).reshape(112, 3)
    out['w_lo'] = _bf(wlo)
    out['b_lo'] = np.ascontiguousarray(np.repeat(
        np.asarray(inp['b_out'], np.float32), 32)[:, None])

    for name, gs in [('ones8', 8), ('ones4', 4), ('ones2', 2)]:
        m = np.zeros((128, 128), np.float32)
        for i in range(128):
            blk = i // gs
            m[gs * blk:gs * blk + gs, i] = 1.0 / gs
        out[name] = m

    out['iota_tile'] = _bf(np.broadcast_to((np.arange(1024) % 32)[None, :],
                                           (128, 1024)).copy())
    out['iota32'] = (np.arange(128)[:, None] % 32).astype(np.float32)
    ids_i = np.asarray(inp['instance_map']).reshape(-1).astype(np.int64)
    cnt = np.bincount(ids_i, minlength=32).astype(np.float32)
    out['rcnt'] = np.ascontiguousarray(
        (1.0 / np.maximum(cnt, 1.0))[:, None])
    return out


def input_specs():
    return {
        'x_pad': ((3, 520, 518), BF16),
        'ids': ((P,), BF16),
        'w_l0': ((4, 84, 128), BF16),
        'w_d0': ((9, 48, 128), BF16),
        'w_d1': ((5, 96, 128), BF16),
        'w_d2': ((2, 3, 96, 128), BF16),
        'w_d3': ((2, 9, 128, 128), BF16),
        'w_u0': ((2, 2, 2, 2, 2, 128, 128), BF16),
        'w_u1': ((2, 2, 2, 128, 128), BF16),
        'w_u2': ((2, 2, 3, 64, 128), BF16),
        'w_u3': ((2, 5, 64, 128), BF16),
        'w_lo': ((38, 112, 96), BF16),
        'b_lo': ((96, 1), F32),
        'ones8': ((128, 128), F32),
        'ones4': ((128, 128), F32),
        'ones2': ((128, 128), F32),
        'iota_tile': ((128, 1024), BF16),
        'iota32': ((128, 1), F32),
        'rcnt': ((32, 1), F32),
    }


# SBUF weight layouts: (sbuf_shape, einops from DRAM shape)
WSPEC = {
    'w_l0': ((84, 4, 128), "j k m -> k j m"),
    'w_d0': ((48, 9, 128), "t k m -> k t m"),
    'w_d1': ((96, 5, 128), "t k m -> k t m"),
    'w_d2': ((96, 2, 3, 128), "h t k m -> k h t m"),
    'w_d3': ((128, 2, 9, 128), "h t k m -> k h t m"),
    'w_u0': ((128, 2, 2, 2, 2, 2, 128), "h a b va vb k m -> k h a b va vb m"),
    'w_u1': ((128, 2, 2, 2, 128), "b t o k m -> k b t o m"),
    'w_u2': ((64, 2, 2, 3, 128), "h b t k m -> k h b t m"),
    'w_u3': ((64, 2, 5, 128), "b t k m -> k b t m"),
    'w_lo': ((112, 38, 96), "t k m -> k t m"),
    'b_lo': ((96, 1), None),
    'ones8': ((128, 128), None),
    'ones4': ((128, 128), None),
    'ones2': ((128, 128), None),
    'iota_tile': ((128, 1024), None),
    'iota32': ((128, 1), None),
    'rcnt': ((32, 1), None),
}


def _inorm_relu(nc, sm, pp, interior, chunks, ones_lhs):
    """In-place instance-norm + relu. chunks: 2D APs [128, <=512]."""
    nchunk = len(chunks)
    stats = sm.tile([128, nchunk, 6], F32, tag="in_stats")
    for i, ch in enumerate(chunks):
        nc.vector.bn_stats(out=stats[:, i, :], in_=ch)
    mv = sm.tile([128, 2], F32, tag="in_mv")
    nc.vector.bn_aggr(out=mv, in_=stats)
    if ones_lhs is not None:
        m3 = sm.tile([128, 3], F32, tag="in_m3")
        nc.vector.tensor_copy(out=m3[:, 0:2], in_=mv)
        nc.vector.tensor_mul(m3[:, 2:3], mv[:, 0:1], mv[:, 0:1])
        cps = pp.tile([128, 3], F32, tag="in_comb")
        nc.tensor.matmul(cps, lhsT=ones_lhs, rhs=m3, start=True, stop=True)
        mbar = sm.tile([128, 3], F32, tag="in_mbar")
        nc.scalar.copy(out=mbar, in_=cps)
        m_col = mbar[:, 0:1]
        var = sm.tile([128, 1], F32, tag="in_var")
        nc.vector.tensor_add(var, mbar[:, 1:2], mbar[:, 2:3])
        mm = sm.tile([128, 1], F32, tag="in_mm")
        nc.vector.tensor_mul(mm, m_col, m_col)
        nc.vector.tensor_sub(var, var, mm)
    else:
        m_col = mv[:, 0:1]
        var = sm.tile([128, 1], F32, tag="in_var")
        nc.vector.tensor_copy(out=var, in_=mv[:, 1:2])
    nc.vector.tensor_scalar_add(var, var, EPS)
    sd = sm.tile([128, 1], F32, tag="in_sd")
    nc.scalar.sqrt(sd, var)
    s_col = sm.tile([128, 1], F32, tag="in_s")
    nc.vector.reciprocal(s_col, sd)
    b_col = sm.tile([128, 1], F32, tag="in_b")
    nc.vector.tensor_scalar(b_col, m_col, s_col, -1.0, AL.mult, AL.mult)
    nc.scalar.activation(interior, interior, AF.Relu, bias=b_col, scale=s_col)


def build(nc, tc, ctx, upto='seg', dbg=None):
    spec = input_specs()
    din = {k: nc.dram_tensor(k, s, d, kind="ExternalInput")
           for k, (s, d) in spec.items()}
    dbg = dbg or {}
    stages = ['h0', 'd0', 'd1', 'd2', 'd3', 'u0', 'u1', 'u2', 'u3', 'f',
              'sums', 'seg']
    sidx = stages.index(upto)
    out_d = nc.dram_tensor("out", (32, 12, 2048), BF16, kind="ExternalOutput")

    sm = ctx.enter_context(tc.tile_pool(name="small", bufs=2))
    acts = ctx.enter_context(tc.tile_pool(name="acts", bufs=1))
    pp_s = ctx.enter_context(tc.tile_pool(name="psum_s", bufs=1, space="PSUM"))
    wpool = ctx.enter_context(tc.tile_pool(name="weights", bufs=1))
    segp = ctx.enter_context(tc.tile_pool(name="segbig", bufs=1))
    # f4Tx[x%128, xc, 96*grp + c*32 + dy] = f[c, 32*grp+dy, 128*xc+x]
    f4Tx = segp.tile([128, 4, 1536], BF16)
    idsTx = segp.tile([128, 4, 512], BF16)
    ids2d = din['ids'].rearrange("(y x) -> y x", x=512)
    for xc in range(4):
        eng = nc.sync if xc % 2 == 0 else nc.sync
        eng.dma_start(out=idsTx[:, xc, :],
                      in_=ids2d[:, 128 * xc:128 * xc + 128], transpose=True)

    wl_ctr = [0]

    def wload(name, pool=None):
        shape, rs = WSPEC[name]
        t = (pool or wpool).tile(list(shape), spec[name][1], tag="w_" + name)
        src = din[name][:]
        if rs is not None:
            src = src.rearrange(rs)
        eng = nc.sync if wl_ctr[0] % 2 == 0 else nc.sync
        wl_ctr[0] += 1
        eng.dma_start(out=t, in_=src)
        return t

    ones8, ones4, ones2 = wload('ones8'), wload('ones4'), wload('ones2')

    def stage_done(name, tile_ap):
        if name in dbg:
            nc.sync.dma_start(out=dbg[name][:], in_=tile_ap)
        return sidx <= stages.index(name)

    # ================= L0 =================
    h0 = acts.tile([128, 64, 513], BF16, tag="slotA")
    nc.vector.memset(h0[:, :, 0:1], 0.0)
    with tc.tile_pool(name="lp_l0", bufs=3) as hrp, \
         tc.tile_pool(name="pp_l0", bufs=4, space="PSUM") as psp:
        w_l0 = wload('w_l0')
        for gb in range(16):
            # hr84[(tx,c,u), r4, x] = x_pad[c, 32*gb + 4*r4 + u, tx + x]
            hr = hrp.tile([84, 10, 512], BF16, tag="hr")
            for tx in range(7):
                for c in range(3):
                    nc.sync.dma_start(
                        out=hr[12 * tx + 4 * c:12 * tx + 4 * c + 4, :, :],
                        in_=din['x_pad'][c][32 * gb:32 * gb + 40,
                                            tx:tx + 512].rearrange(
                            "(r u) x -> u r x", u=4))
            for g in range(4):
                ps = psp.tile([128, 512], F32, tag="ps")
                for j in range(4):
                    nc.tensor.matmul(ps, lhsT=w_l0[:, j, :],
                                     rhs=hr[:, 2 * g + j, :],
                                     start=(j == 0), stop=(j == 3))
                if g % 2 == 0:
                    nc.scalar.copy(out=h0[:, 4 * gb + g, 1:513], in_=ps)
                else:
                    nc.vector.tensor_copy(out=h0[:, 4 * gb + g, 1:513], in_=ps)
    _inorm_relu(nc, sm, pp_s, h0[:, :, 1:513],
                [h0[:, i, 1:513] for i in range(64)], ones8)
    if stage_done('h0', h0):
        return din

    # ================= D0 =================
    d0 = acts.tile([128, 64, 257], BF16, tag="slotB")
    nc.vector.memset(d0[:, :, 0:1], 0.0)
    h0r = h0.rearrange("(c d) g x -> c d g x", d=8)
    with tc.tile_pool(name="lp_d0", bufs=2) as hrp, \
         tc.tile_pool(name="pp_d0", bufs=4, space="PSUM") as psp:
        w_d0 = wload('w_d0', hrp)
        for gb in range(16):
            hr = hrp.tile([48, 33, 511], BF16, tag="hr")
            hr4 = hr.rearrange("(c t) r x -> t c r x", t=3)
            if gb == 0:
                nc.vector.memset(hr[:, 0:1, :], 0.0)
            for tx in range(3):
                for d in range(8):
                    rr0 = (d + 1) % 8
                    ks = 1 if (gb == 0 and d == 7) else 0
                    rows = [rr0 + 8 * k for k in range(ks, (33 - rr0 + 7) // 8)]
                    g0 = (32 * gb + rows[0] - 1) // 8
                    nc.sync.dma_start(
                        out=hr4[tx][:, rows[0]:rows[-1] + 1:8, :],
                        in_=h0r[:, d, g0:g0 + len(rows), tx:tx + 511])
            for blk in range(2):
                ps = psp.tile([128, 2, 256], F32, tag="ps")
                for t in range(9):
                    s0 = 16 * blk + t
                    rhs = hr[:, s0:s0 + 9:8, 0:511:2]
                    nc.tensor.matmul(ps, lhsT=w_d0[:, t, :], rhs=rhs,
                                     start=(t == 0), stop=(t == 8))
                if blk % 2 == 0:
                    nc.scalar.copy(
                        out=d0[:, 4 * gb + 2 * blk:4 * gb + 2 * blk + 2, 1:257],
                        in_=ps)
                else:
                    nc.vector.tensor_copy(
                        out=d0[:, 4 * gb + 2 * blk:4 * gb + 2 * blk + 2, 1:257],
                        in_=ps)
    _inorm_relu(nc, sm, pp_s, d0[:, :, 1:257],
                [d0[:, i, 1:257] for i in range(64)], ones4)
    if stage_done('d0', d0):
        return din

    # ================= D1 =================
    d1 = acts.tile([128, 64, 129], BF16, tag="slotA")
    nc.vector.memset(d1[:, :, 0:1], 0.0)
    d0r = d0.rearrange("(c d) g x -> c d g x", d=4)
    with tc.tile_pool(name="lp_d1", bufs=2) as hrp, \
         tc.tile_pool(name="pp_d1", bufs=4, space="PSUM") as psp:
        w_d1 = wload('w_d1', hrp)
        for gb in range(8):
            hr = hrp.tile([96, 33, 255], BF16, tag="hr")
            hr4 = hr.rearrange("(c t) r x -> t c r x", t=3)
            if gb == 0:
                nc.vector.memset(hr[:, 0:1, :], 0.0)
            for tx in range(3):
                for d in range(4):
                    rr0 = (d + 1) % 4
                    ks = 1 if (gb == 0 and d == 3) else 0
                    rows = [rr0 + 4 * k for k in range(ks, (33 - rr0 + 3) // 4)]
                    g0 = (32 * gb + rows[0] - 1) // 4
                    eng = nc.sync if (d % 2 == 0) else nc.sync
                    eng.dma_start(
                        out=hr4[tx][:, rows[0]:rows[-1] + 1:4, :],
                        in_=d0r[:, d, g0:g0 + len(rows), tx:tx + 255])
            for blk in range(2):
                ps = psp.tile([128, 4, 128], F32, tag="ps")
                for t in range(5):
                    s0 = 16 * blk + t
                    rhs = hr[:, s0:s0 + 13:4, 0:255:2]
                    nc.tensor.matmul(ps, lhsT=w_d1[:, t, :], rhs=rhs,
                                     start=(t == 0), stop=(t == 4))
                if blk % 2 == 0:
                    nc.scalar.copy(
                        out=d1[:, 8 * gb:8 * gb + 4, 1:129], in_=ps)
                else:
                    nc.vector.tensor_copy(
                        out=d1[:, 8 * gb + 4:8 * gb + 8, 1:129], in_=ps)
    _inorm_relu(nc, sm, pp_s, d1[:, :, 1:129],
                [d1[:, i, 1:129] for i in range(64)], ones2)
    if stage_done('d1', d1):
        return din

    # ================= D2 =================
    d2 = acts.tile([128, 65, 65], BF16, tag="slotB")
    nc.vector.memset(d2[:, 0:1, :], 0.0)
    nc.vector.memset(d2[:, :, 0:1], 0.0)
    d1r = d1.rearrange("(c d) g x -> c d g x", d=2)
    with tc.tile_pool(name="lp_d2", bufs=1) as hrp, \
         tc.tile_pool(name="pp_d2", bufs=4, space="PSUM") as psp:
        w_d2 = wload('w_d2', hrp)
        for gb in range(2):
            hr = hrp.tile([96, 2, 65, 127], BF16, tag="hr")
            hr4 = hr.rearrange("(c t) h r x -> t c h r x", t=3)
            if gb == 0:
                nc.vector.memset(hr[:, :, 0:1, :], 0.0)
            for h in range(2):
                for tx in range(3):
                    for d in range(2):
                        rr0 = (d + 1) % 2
                        ks = 1 if (gb == 0 and d == 1) else 0
                        rows = [rr0 + 2 * k
                                for k in range(ks, (65 - rr0 + 1) // 2)]
                        g0 = (64 * gb + rows[0] - 1) // 2
                        eng = nc.sync if ((h + tx) % 2 == 0) else nc.sync
                        eng.dma_start(
                            out=hr4[tx][:, h, rows[0]:rows[-1] + 1:2, :],
                            in_=d1r[32 * h:32 * h + 32, d, g0:g0 + len(rows),
                                    tx:tx + 127])
            for blk in range(4):
                ps = psp.tile([128, 8, 64], F32, tag="ps")
                first = True
                for h in range(2):
                    for t in range(3):
                        s0 = 16 * blk + t
                        rhs = hr[:, h, s0:s0 + 15:2, 0:127:2]
                        nc.tensor.matmul(ps, lhsT=w_d2[:, h, t, :], rhs=rhs,
                                         start=first, stop=(h == 1 and t == 2))
                        first = False
                if blk % 2 == 0:
                    nc.scalar.copy(
                        out=d2[:, 1 + 32 * gb + 8 * blk:
                               1 + 32 * gb + 8 * blk + 8, 1:65],
                        in_=ps)
                else:
                    nc.vector.tensor_copy(
                        out=d2[:, 1 + 32 * gb + 8 * blk:
                               1 + 32 * gb + 8 * blk + 8, 1:65],
                        in_=ps)
    _inorm_relu(nc, sm, pp_s, d2[:, 1:65, 1:65],
                [d2[:, 1 + i, 1:65] for i in range(64)], None)
    if stage_done('d2', d2):
        return din

    # ================= D3 =================
    d3 = acts.tile([128, 2, 33, 33], BF16, tag="slotA")
    nc.vector.memset(d3[:, :, 32:33, :], 0.0)
    nc.vector.memset(d3[:, :, :, 32:33], 0.0)
    with tc.tile_pool(name="lp_d3", bufs=1) as hrp, \
         tc.tile_pool(name="pp_d3", bufs=4, space="PSUM") as psp:
        w_d3 = wload('w_d3', hrp)
        for h in range(2):
            for blk in range(2):
                ps = psp.tile([128, 16, 32], F32, tag="ps")
                first = True
                for ky in range(3):
                    for kx in range(3):
                        s0 = 32 * blk + ky
                        rhs = d2[:, s0:s0 + 31:2, kx:kx + 63:2]
                        nc.tensor.matmul(ps, lhsT=w_d3[:, h, ky * 3 + kx, :],
                                         rhs=rhs, start=first,
                                         stop=(ky == 2 and kx == 2))
                        first = False
                nc.scalar.copy(out=d3[:, h, 16 * blk:16 * blk + 16, 0:32], in_=ps)
    for h in range(2):
        _inorm_relu(nc, sm, pp_s, d3[:, h, 0:32, 0:32],
                    [d3[:, h, i, 0:32] for i in range(32)], None)
    if stage_done('d3', d3):
        return din

    # ================= U0 =================
    u0 = acts.tile([128, 65, 65], BF16, tag="slotB")
    nc.vector.memset(u0[:, 64:65, :], 0.0)
    nc.vector.memset(u0[:, :, 64:65], 0.0)
    with tc.tile_pool(name="lp_u0", bufs=1) as hrp, \
         tc.tile_pool(name="pp_u0", bufs=4, space="PSUM") as psp:
        w_u0 = wload('w_u0', hrp)
        for a in range(2):
            for b in range(2):
                for blk in range(2):
                    ps = psp.tile([128, 16, 32], F32, tag="ps")
                    mms = [(h, va, vb) for h in range(2) for va in range(2)
                           for vb in range(2)
                           if 0 <= a + 1 - 2 * va < 3 and 0 <= b + 1 - 2 * vb < 3]
                    for mi, (h, va, vb) in enumerate(mms):
                        rhs = d3[:, h, 16 * blk + va:16 * blk + va + 16,
                                 vb:vb + 32]
                        nc.tensor.matmul(ps, lhsT=w_u0[:, h, a, b, va, vb, :],
                                         rhs=rhs, start=(mi == 0),
                                         stop=(mi == len(mms) - 1))
                    nc.scalar.copy(
                        out=u0[:, 32 * blk + a:32 * blk + a + 31:2, b:b + 63:2],
                        in_=ps)
    _inorm_relu(nc, sm, pp_s, u0[:, 0:64, 0:64],
                [u0[:, i, 0:64] for i in range(64)], None)
    if stage_done('u0', u0):
        return din

    # ================= U1 =================
    u1 = acts.tile([128, 65, 65, 2], BF16, tag="slotA")
    nc.vector.memset(u1[:, 64:65, :, :], 0.0)
    nc.vector.memset(u1[:, :, 64:65, :], 0.0)
    with tc.tile_pool(name="lp_u1", bufs=1) as hrp, \
         tc.tile_pool(name="pp_u1", bufs=4, space="PSUM") as psp:
        w_u1 = wload('w_u1', hrp)
        for b in range(2):
            for blk in range(8):
                ps = psp.tile([128, 8, 64], F32, tag="ps")
                mms = [(t, hoff) for t in range(2) for hoff in range(2)]
                for mi, (t, hoff) in enumerate(mms):
                    rhs = u0[:, 8 * blk + t:8 * blk + t + 8, hoff:hoff + 64]
                    nc.tensor.matmul(ps, lhsT=w_u1[:, b, t, hoff, :], rhs=rhs,
                                     start=(mi == 0), stop=(mi == len(mms) - 1))
                if blk % 2 == 0:
                    nc.scalar.copy(out=u1[:, 8 * blk:8 * blk + 8, 0:64, b],
                                   in_=ps)
                else:
                    nc.vector.tensor_copy(
                        out=u1[:, 8 * blk:8 * blk + 8, 0:64, b], in_=ps)
    u1x = u1.rearrange("p g x b -> p g (x b)")
    _inorm_relu(nc, sm, pp_s, u1x[:, 0:64, 0:128],
                [u1x[:, i, 0:128] for i in range(64)], ones2)
    if stage_done('u1', u1):
        return din

    # ================= U2 =================
    u2 = acts.tile([128, 65, 129, 2], BF16, tag="slotB")
    nc.vector.memset(u2[:, 64:65, :, :], 0.0)
    nc.vector.memset(u2[:, :, 128:129, :], 0.0)
    with tc.tile_pool(name="lp_u2", bufs=1) as hrp, \
         tc.tile_pool(name="pp_u2", bufs=4, space="PSUM") as psp:
        w_u2 = wload('w_u2', hrp)
        u1rA = hrp.tile([64, 129, 129], BF16, tag="u1repA")
        u1rB = hrp.tile([64, 129, 129], BF16, tag="u1repB")
        u1rh = [u1rA, u1rB]
        for h in range(2):
            u1rr = u1rh[h].rearrange("(c o) r x -> o c r x", o=2)
            for off in range(2):
                for df in range(2):
                    cnt = 65 if df == 0 else 64
                    eng = nc.sync if ((h + off) % 2 == 0) else nc.sync
                    eng.dma_start(
                        out=u1rr[off, :, df:df + 2 * cnt - 1:2, :],
                        in_=u1x[64 * h + df:64 * h + df + 63:2, 0:cnt,
                                off:off + 129])
        for b in range(2):
            for blk in range(16):
                ps = psp.tile([128, 4, 128], F32, tag="ps")
                first = True
                for h in range(2):
                    for t in range(3):
                        s0 = 8 * blk + t
                        rhs = u1rh[h][:, s0:s0 + 7:2, 0:128]
                        nc.tensor.matmul(ps, lhsT=w_u2[:, h, b, t, :], rhs=rhs,
                                         start=first, stop=(h == 1 and t == 2))
                        first = False
                if blk % 2 == 0:
                    nc.scalar.copy(out=u2[:, 4 * blk:4 * blk + 4, 0:128, b],
                                   in_=ps)
                else:
                    nc.vector.tensor_copy(
                        out=u2[:, 4 * blk:4 * blk + 4, 0:128, b], in_=ps)
    u2x = u2.rearrange("p g x b -> p g (x b)")
    _inorm_relu(nc, sm, pp_s, u2x[:, 0:64, 0:256],
                [u2x[:, i, 0:256] for i in range(64)], ones4)
    if stage_done('u2', u2):
        return din

    # ================= U3 =================
    u3 = acts.tile([128, 64, 260, 2], BF16, tag="slotA")
    u3X = u3.rearrange("p g x b -> p g (x b)")
    w_u3 = wload('w_u3')
    with tc.tile_pool(name="lp_u3", bufs=2) as hrp, \
         tc.tile_pool(name="pp_u3", bufs=4, space="PSUM") as psp:
        u2q = u2x.rearrange("(c d) g x -> c d g x", d=4)
        for gb in range(4):
            u2r = hrp.tile([64, 65, 257], BF16, tag="u2rep")
            u2rr = u2r.rearrange("(c o) r x -> o c r x", o=2)
            for off in range(2):
                for d in range(4):
                    cnt = (65 - d + 3) // 4
                    nc.sync.dma_start(
                        out=u2rr[off, :, d:d + 4 * cnt - 3:4, :],
                        in_=u2q[:, d, 16 * gb:16 * gb + cnt, off:off + 257])
            for b in range(2):
                for blkl in range(8):
                    ps = psp.tile([128, 2, 256], F32, tag="ps")
                    for t in range(5):
                        s0 = 8 * blkl + t
                        rhs = u2r[:, s0:s0 + 5:4, 0:256]
                        nc.tensor.matmul(ps, lhsT=w_u3[:, b, t, :], rhs=rhs,
                                         start=(t == 0), stop=(t == 4))
                    g3 = 16 * gb + 2 * blkl
                    if blkl % 2 == 0:
                        nc.scalar.copy(out=u3[:, g3:g3 + 2, 2:258, b], in_=ps)
                    else:
                        nc.vector.tensor_copy(out=u3[:, g3:g3 + 2, 2:258, b],
                                              in_=ps)
    _inorm_relu(nc, sm, pp_s, u3X[:, :, 4:516],
                [u3X[:, i, 4:516] for i in range(64)], ones8)
    for dst, src in [(3, 5), (2, 6), (1, 7), (516, 514), (517, 513), (518, 512)]:
        nc.scalar.copy(out=u3X[:, :, dst:dst + 1], in_=u3X[:, :, src:src + 1])
    if stage_done('u3', u3):
        return din

    # ================= L_out =================
    f_nat = acts.tile([96, 16, 512], BF16, tag="slotB")
    f_d2 = nc.dram_tensor("f_d2", (16, 96, 512), BF16, kind="Internal")
    # u3d[c, y+3, x] = relu(inorm(u3))[c, y, x] with 3-row reflect pads baked
    # in; x dim = u3X cols 1..518 (x-reflect cols already materialized).
    u3d = nc.dram_tensor("u3d", (16, 518, 518), BF16, kind="Internal")
    for c in range(16):
        eng = nc.sync if c % 2 == 0 else nc.sync
        eng.dma_start(
            out=u3d[c][3:515, :].rearrange("(g b) x -> b g x", b=8),
            in_=u3X[8 * c:8 * c + 8, :, 1:519])
    for r, y in [(0, 3), (1, 2), (2, 1), (515, 510), (516, 509), (517, 508)]:
        eng = nc.sync if r % 2 == 0 else nc.sync
        eng.dma_start(out=u3d[:, r, :],
                      in_=u3X[y % 8:y % 8 + 121:8, y // 8, 1:519])
    w_lo = wload('w_lo')
    b_lo = wload('b_lo')
    iota_tile = wload('iota_tile', segp)
    iota32 = wload('iota32', segp)
    rcnt = wload('rcnt', segp)
    ohp = ctx.enter_context(tc.tile_pool(name="segoh", bufs=3))
    psum_s2 = pp_s.tile([3, 32], F32, tag="segsum")
    iota_3d = iota_tile[:, 0:1024].rearrange("p (b k) -> p b k", k=32)
    with tc.tile_pool(name="lp_lo", bufs=2) as hrp, \
         tc.tile_pool(name="pp_lo", bufs=4, space="PSUM") as psp:
        for grp in range(16):
            hrA = hrp.tile([112, 19, 512], BF16, tag="hr")
            hrB = hrp.tile([112, 19, 512], BF16, tag="hr")
            hA4 = hrA.rearrange("(c t) r x -> t c r x", t=7)
            hB4 = hrB.rearrange("(c t) r x -> t c r x", t=7)
            for tx in range(7):
                eng = nc.sync if (grp + tx) % 2 == 0 else nc.sync
                eng.dma_start(
                    out=hA4[tx],
                    in_=u3d[:, 32 * grp:32 * grp + 19, tx:tx + 512])
                eng.dma_start(
                    out=hB4[tx],
                    in_=u3d[:, 32 * grp + 19:32 * grp + 38, tx:tx + 512])
            ps = psp.tile([96, 512], F32, tag="ps")
            for t in range(38):
                rhs = hrA[:, t, :] if t < 19 else hrB[:, t - 19, :]
                nc.tensor.matmul(ps, lhsT=w_lo[:, t, :], rhs=rhs,
                                 start=(t == 0), stop=(t == 37))
            nc.scalar.activation(f_nat[:, grp, :], ps, AF.Tanh, bias=b_lo,
                                 scale=1.0)
            for xc in range(4):
                nc.sync.dma_start(
                    out=f4Tx[:, xc, 96 * grp:96 * grp + 96],
                    in_=f_nat[:, grp, 128 * xc:128 * xc + 128],
                    transpose=True)
                oh = ohp.tile([128, 32, 32], BF16)
                nc.vector.tensor_tensor(
                    out=oh,
                    in0=idsTx[:, xc, 32 * grp:32 * grp + 32].unsqueeze(2)
                    .broadcast_to([128, 32, 32]),
                    in1=iota_3d, op=AL.is_equal)
                for yi in range(32):
                    base = 96 * grp + yi
                    nc.tensor.matmul(psum_s2,
                                     lhsT=f4Tx[:, xc, base:base + 65:32],
                                     rhs=oh[:, yi, :],
                                     start=(grp == 0 and xc == 0 and yi == 0),
                                     stop=(grp == 15 and xc == 3 and yi == 31),
                                     skip_group_check=True)
    if stage_done('f', f_nat):
        return din

    # ================= segment mean =================
    gat = ctx.enter_context(tc.tile_pool(name="seggat", bufs=3))
    ppg = ctx.enter_context(tc.tile_pool(name="psumg", bufs=4, space="PSUM"))
    if 'f4Tx' in dbg:
        nc.sync.dma_start(out=dbg['f4Tx'][:], in_=f4Tx[:])
    if 'idsTx' in dbg:
        nc.sync.dma_start(out=dbg['idsTx'][:], in_=idsTx[:])

    sums32 = sm.tile([32, 32], F32, tag="sums32")
    nc.vector.memset(sums32, 0.0)
    nc.scalar.copy(out=sums32[0:3, :], in_=psum_s2)
    sumsT = sm.tile([32, 32], F32, tag="sumsT")
    nc.vector.transpose(sumsT, sums32)
    means_bf = sm.tile([32, 3], BF16, tag="means_bf")
    nc.vector.tensor_scalar_mul(means_bf, sumsT[:, 0:3], rcnt)
    bd = sm.tile([128, 12], BF16, tag="bd")
    nc.vector.memset(bd, 0.0)
    for s in range(4):
        nc.sync.dma_start(out=bd[32 * s:32 * s + 32, 3 * s:3 * s + 3],
                          in_=means_bf)

    if sidx <= stages.index('sums'):
        return din
    ids_q = din['ids'].rearrange("(q n) -> q n", q=4)
    for t in range(32):
        ids_rep = gat.tile([128, 2048], BF16)
        for q in range(4):
            eng = nc.sync if q % 2 == 0 else nc.sync
            eng.dma_start(
                out=ids_rep[32 * q:32 * q + 32, :],
                in_=ids_q[q:q + 1, t * 2048:(t + 1) * 2048].broadcast_to(
                    [32, 2048]))
        oh_g = gat.tile([128, 2048], BF16)
        nc.vector.tensor_scalar(out=oh_g, in0=ids_rep, scalar1=iota32,
                                scalar2=None, op0=AL.is_equal)
        stg = gat.tile([12, 2048], BF16)
        for w in range(4):
            psg = ppg.tile([12, 512], F32)
            nc.tensor.matmul(psg, lhsT=bd, rhs=oh_g[:, 512 * w:512 * w + 512],
                             start=True, stop=True)
            if w % 2 == 0:
                nc.vector.tensor_copy(out=stg[:, 512 * w:512 * w + 512],
                                      in_=psg)
            else:
                nc.scalar.copy(out=stg[:, 512 * w:512 * w + 512], in_=psg)
        nc.sync.dma_start(out=out_d[t], in_=stg)
    return din


# ======================================================================
# public entry: kernel(**inputs) with FULL batch inputs, 8-core SPMD
# ======================================================================
import concourse.bacc as _bacc
from concourse import bass_utils as _bass_utils

_CACHE = {}


def _get_nc():
    if 'nc' not in _CACHE:
        nc = _bacc.Bacc("TRN2", target_bir_lowering=False)
        with contextlib.ExitStack() as ctx:
            tc = ctx.enter_context(tile.TileContext(nc, pool_alloc_mode="queue"))
            build(nc, tc, ctx, upto='seg')
        nc.compile()
        _CACHE['nc'] = nc
    return _CACHE['nc']


def kernel(**inputs):
    nc = _get_nc()
    x = np.asarray(inputs['x'])
    ids = np.asarray(inputs['instance_map'])
    B = x.shape[0]
    shared = None
    in_maps = []
    for bi in range(B):
        inp0 = {k: v for k, v in inputs.items()}
        inp0['x'] = x[bi]
        inp0['instance_map'] = ids[bi]
        if shared is None:
            m = pack_inputs(inp0)
            shared = {k: v for k, v in m.items()
                      if k not in ('x_pad', 'ids', 'rcnt')}
        else:
            m = dict(shared)
            xp = np.pad(np.asarray(inp0['x'], np.float32), ((0, 0), (3, 3), (3, 3)),
                        mode='reflect')
            m['x_pad'] = _bf(np.pad(xp, ((0, 0), (0, 2), (0, 0))))
            ids_i = np.asarray(inp0['instance_map']).reshape(-1)
            m['ids'] = _bf(ids_i.astype(np.float32))
            cnt = np.bincount(ids_i.astype(np.int64),
                              minlength=32).astype(np.float32)
            m['rcnt'] = np.ascontiguousarray(
                (1.0 / np.maximum(cnt, 1.0))[:, None])
        in_maps.append(m)
    res = _bass_utils.run_bass_kernel_spmd(nc, in_maps, core_ids=list(range(B)))
    out = np.stack([_unpack_out(res.results[i]['out']) for i in range(B)])
    return out.astype(np.float32)


def _unpack_out(a):
    a = np.asarray(a).astype(np.float32).reshape(32, 4, 3, 2048)
    return a.transpose(2, 1, 0, 3).reshape(3, 512, 512)


def kernel_traced(**inputs):
    """Like kernel() but with NTFF tracing; returns (out, exec_time_ns, profile)."""
    nc = _get_nc()
    x = np.asarray(inputs['x'])
    ids = np.asarray(inputs['instance_map'])
    B = x.shape[0]
    shared = None
    in_maps = []
    for bi in range(B):
        inp0 = {k: v for k, v in inputs.items()}
        inp0['x'] = x[bi]
        inp0['instance_map'] = ids[bi]
        if shared is None:
            m = pack_inputs(inp0)
            shared = {k: v for k, v in m.items()
                      if k not in ('x_pad', 'ids', 'rcnt')}
        else:
            m = dict(shared)
            xp = np.pad(np.asarray(inp0['x'], np.float32), ((0, 0), (3, 3), (3, 3)),
                        mode='reflect')
            m['x_pad'] = _bf(np.pad(xp, ((0, 0), (0, 2), (0, 0))))
            ids_i = np.asarray(inp0['instance_map']).reshape(-1)
            m['ids'] = _bf(ids_i.astype(np.float32))
            cnt = np.bincount(ids_i.astype(np.int64),
                              minlength=32).astype(np.float32)
            m['rcnt'] = np.ascontiguousarray(
                (1.0 / np.maximum(cnt, 1.0))[:, None])
        in_maps.append(m)
    res = _bass_utils.run_bass_kernel_spmd(nc, in_maps, core_ids=list(range(B)),
                                           trace=True)
    out = np.stack([_unpack_out(res.results[i]['out']) for i in range(B)])
    return out.astype(np.float32), res.exec_time_ns, res

